# revision 12
# baseline (speedup 1.0000x reference)
"""Cumulative VWAP kernel for Trainium2 (Bass/Tile), data-parallel over 8 cores.

vwap[:, t] = cumsum(s*v)[:, t] / (cumsum(v)[:, t] + 1e-8),  vwap[:, 0] = s[:, 0]

Sharding: num_paths (axis 0) split evenly across 8 NeuronCores; the cumsum
runs along the time axis, which stays local to each core (no collectives).

Key engine facts driving the design (all HW-measured on trn2):
  * The builtin DVE tensor_tensor_scan runs at ~2 cycles/element (two ALU ops
    in the recurrence).  A custom DVE op (dve_spec `scan()` node) runs at
    ~1 element/cycle and can fuse the s*v multiply into the scan input
    (feed-forward stage), so pv_cum costs one op and no separate multiply.
  * GpSimd cannot run scans (ISA rejects opcode 0xe5 on Pool) and serializes
    against DVE scans on the shared SBUF port pair -> GpSimd unused.
  * ACT Reciprocal is banned (accuracy); 1/x = exp(-ln(x)) on ACT instead,
    with both functions forced into ONE activation-table set (otherwise the
    table-load pass alternates natural_log/exp_and_others every tile,
    ~2.7us per reload).
  * eps=1e-8 is a fp32 no-op since v_cum >= 1e6 (ulp >= 1/16).

Per-core dataflow, per [128, 4096] tile (16 tiles per core):
  DMA  : load stock tile, load volume tile          (2 MiB each, contiguous)
  ACT  : save col 0 of stock (the t==0 fix is an exact copy of s0)
  DVE  : pv_cum = custom scan(ADD, s*v)             (~4.3 us, in place)
  DVE  : v_cum  = custom scan(ADD, v)               (~4.2 us, in place)
  ACT  : r = exp(-ln(v_cum))                        (2 ops, one table set)
  DVE  : vwap = pv_cum * r                          (tensor_tensor, ~4.4 us)
  ACT  : restore col 0
  DMA  : store vwap tile
Engine busy per core: DVE ~220us, ACT ~130us, DMA ~270us -> DMA-bound.
"""

import numpy as np

NUM_PATHS = 16384
TIME = 4096
N_CORES = 8
ROWS = NUM_PATHS // N_CORES  # rows per core
P = 128  # SBUF partitions

_CACHE = {}

_COMBINED_SET = "natural_log_exp_and_others"


def _single_act_set_bacc():
    import concourse.bacc as bacc

    class SingleActSetBacc(bacc.Bacc):
        """Restrict the activation-table-load pass to one set holding
        Ln+Exp+Copy so alternating Ln/Exp doesn't reload tables every tile."""

        def insert_act_table_loads(self):
            import bass_rust
            import concourse.mybir as mybir
            from concourse.hw_specs import get_activation_tables

            has_activation = any(
                isinstance(i, mybir.InstActivation)
                for b in self.main_func.blocks
                for i in b.instructions
            )
            if not has_activation:
                return
            tables = [
                (name, fns if name == _COMBINED_SET else set())
                for name, fns in get_activation_tables(self.m.arch).items()
            ]
            bass_rust.insert_act_table_loads(self, tables)

    return SingleActSetBacc


def _register_custom_ops():
    """Register the two custom DVE cumsum ops (idempotent)."""
    import concourse.dve_ops as dve_ops
    from concourse.dve_ops import DveOp
    from concourse.dve_spec import (
        AluOp, C0, Spec, Src0, Src1, lower, scan, spec_leaves,
    )
    from concourse.dve_uop import DveOpSpec

    def register(name, spec):
        for o in dve_ops.OPS:
            if o.name == name:
                return o
        op = DveOp(name, spec, subdim=False, uops_sha={})
        dve_ops.OPS.append(op)
        dve_ops.CUSTOM_DVE_SPECS[name] = spec
        dve_ops._SUB_OPCODE_FOR_NAME[name] = (
            dve_ops._CUSTOM_DVE_ROW_BASE + len(dve_ops.OPS) - 1
        )
        assert dve_ops._SUB_OPCODE_FOR_NAME[name] < 0x20
        # self-pin the uop hashes (same computation DveOp.compile checks)
        for ver in ("v3", "v4"):
            s = DveOpSpec(
                name=name,
                opcode=dve_ops.get_dve_sub_opcode(name),
                uops=lower(spec, ver=ver),
                rd1_en=Src1 in spec_leaves(spec),
            )
            op.uops_sha[ver] = s.sha(ver)
        return op

    pv = register(
        "PV_CUMSUM_ANT",
        Spec(
            body=scan(AluOp.ADD, Src0 * Src1),
            reference=lambda in0, in1, s0, s1, imm2: np.cumsum(
                in0.astype(np.float32) * in1.astype(np.float32),
                axis=-1, dtype=np.float32,
            ),
        ),
    )
    v = register(
        "V_CUMSUM_ANT",
        Spec(
            body=scan(AluOp.ADD, Src0),
            reference=lambda in0, in1, s0, s1, imm2: np.cumsum(
                in0, axis=-1, dtype=np.float32
            ),
        ),
    )
    addmul = register(
        "ADD_MUL_ANT",  # out = (in0 + s0) * in1; s0 is a [P,1] per-partition AP
        Spec(
            body=(Src0 + C0) * Src1,
            reference=lambda in0, in1, s0, s1, imm2: (
                (in0 + s0) * in1
            ).astype(np.float32),
        ),
    )
    return pv, v, addmul


def _build(rows=ROWS, time=TIME, bufs=4, reps=1, n_blk=8):
    """n_blk: scans run per block of time/n_blk columns, with the running
    carry applied via ACT bias (v) / the ADD_MUL custom op (pv).  Shorter
    scan segments cut the fp32 association error of the running sums
    (~proportional to segment length) at no DVE-time cost."""
    import concourse.tile as tile
    import concourse.mybir as mybir

    import concourse.bacc as bacc

    pv_op, v_op, addmul_op = _register_custom_ops()
    # Plain Bacc: the ATL pass alternates the natural_log / exp_and_others
    # table sets (~2.7us per reload), but ACT has headroom under the DMA
    # floor, and the DEDICATED natural_log set's ln spline is ~10x more
    # accurate than the combined natural_log_exp_and_others set's (4.9e-4 vs
    # 5e-5 end-to-end, HW-measured) — accuracy wins here.
    nc = bacc.Bacc("TRN2", target_bir_lowering=False, debug=False)
    f32 = mybir.dt.float32
    stock = nc.dram_tensor("stock_paths", [rows, time], f32, kind="ExternalInput").ap()
    vol = nc.dram_tensor("volume_paths", [rows, time], f32, kind="ExternalInput").ap()
    out = nc.dram_tensor("vwap_out", [rows, time], f32, kind="ExternalOutput").ap()

    Ln = mybir.ActivationFunctionType.Ln
    Exp = mybir.ActivationFunctionType.Exp

    seg = time // n_blk
    n_tiles = rows // P
    with tile.TileContext(nc) as tc:
        with (
            tc.tile_pool(name="big", bufs=bufs) as big,
            tc.tile_pool(name="small", bufs=bufs) as small,
        ):
            for i in range(n_tiles * reps):
                r0 = (i % n_tiles) * P
                ts = big.tile([P, time], f32, tag="ts")
                tv = big.tile([P, time], f32, tag="tv")
                nc.sync.dma_start(ts[:], stock[r0 : r0 + P, :])
                nc.sync.dma_start(tv[:], vol[r0 : r0 + P, :])
                t0 = small.tile([P, 1], f32, tag="t0")
                nc.scalar.copy(t0[:], ts[:, 0:1])
                # block-local scans (pv before v per block: pv reads raw v)
                for b in range(n_blk):
                    sl = slice(b * seg, (b + 1) * seg)
                    nc.vector._custom_dve(pv_op, out=ts[:, sl], in0=ts[:, sl],
                                          in1=tv[:, sl])
                    nc.vector._custom_dve(v_op, out=tv[:, sl], in0=tv[:, sl])
                if n_blk > 1:
                    # carries: inclusive cumsum of the block-end values
                    cp = small.tile([P, n_blk], f32, tag="cp")
                    cv = small.tile([P, n_blk], f32, tag="cv")
                    ends = slice(seg - 1, time, seg)
                    nc.vector.tensor_copy(cp[:], ts[:, ends])
                    nc.vector.tensor_copy(cv[:], tv[:, ends])
                    nc.vector._custom_dve(v_op, out=cp[:], in0=cp[:])
                    nc.vector._custom_dve(v_op, out=cv[:], in0=cv[:])
                # v_cum -> 1/v_cum on ACT, folding the v-carry into ln's bias
                for b in range(n_blk):
                    sl = slice(b * seg, (b + 1) * seg)
                    bias = 0.0 if b == 0 else cv[:, b - 1 : b]
                    nc.scalar.activation(tv[:, sl], tv[:, sl], Ln, bias=bias)
                nc.scalar.activation(tv[:], tv[:], Exp, scale=-1.0)
                # vwap = (pv_cum_block + pv_carry) * r
                for b in range(n_blk):
                    sl = slice(b * seg, (b + 1) * seg)
                    if b == 0:
                        nc.vector.tensor_mul(ts[:, sl], ts[:, sl], tv[:, sl])
                    else:
                        nc.vector._custom_dve(
                            addmul_op, out=ts[:, sl], in0=ts[:, sl],
                            in1=tv[:, sl], s0=cp[:, b - 1 : b],
                        )
                nc.scalar.copy(ts[:, 0:1], t0[:])
                nc.sync.dma_start(out[r0 : r0 + P, :], ts[:])
    nc.compile()
    return nc


def _get_nc():
    if "nc" not in _CACHE:
        _CACHE["nc"] = _build()
    return _CACHE["nc"]


def kernel(stock_paths: np.ndarray, volume_paths: np.ndarray) -> np.ndarray:
    from concourse.bass_utils import run_bass_kernel_spmd

    stock_paths = np.ascontiguousarray(stock_paths, dtype=np.float32)
    volume_paths = np.ascontiguousarray(volume_paths, dtype=np.float32)
    assert stock_paths.shape == (NUM_PATHS, TIME)

    nc = _get_nc()
    in_maps = [
        {
            "stock_paths": stock_paths[i * ROWS : (i + 1) * ROWS],
            "volume_paths": volume_paths[i * ROWS : (i + 1) * ROWS],
        }
        for i in range(N_CORES)
    ]
    res = run_bass_kernel_spmd(nc, in_maps, core_ids=list(range(N_CORES)))
    return np.concatenate([r["vwap_out"] for r in res.results], axis=0)


# revision 28
# speedup vs baseline: 1.2244x; 1.2244x over previous
"""Cumulative VWAP kernel for Trainium2 (Bass/Tile), data-parallel over 8 cores.

vwap[:, t] = cumsum(s*v)[:, t] / (cumsum(v)[:, t] + 1e-8),  vwap[:, 0] = s[:, 0]

Sharding: num_paths (axis 0) split evenly across 8 NeuronCores; the cumsum
runs along the time axis, which stays local to each core (no collectives).

Key engine facts driving the design (all HW-measured on trn2):
  * The builtin DVE tensor_tensor_scan runs at ~2 cycles/element (two ALU ops
    in the recurrence).  A custom DVE op (dve_spec `scan()` node) runs at
    ~1 element/cycle and can fuse the s*v multiply into the scan input
    (feed-forward stage), so pv_cum costs one op and no separate multiply.
  * GpSimd cannot run scans (ISA rejects opcode 0xe5 on Pool) and serializes
    against DVE scans on the shared SBUF port pair -> GpSimd unused.
  * ACT Reciprocal is banned (accuracy); 1/x = exp(-ln(x)) on ACT instead,
    with both functions forced into ONE activation-table set (otherwise the
    table-load pass alternates natural_log/exp_and_others every tile,
    ~2.7us per reload).
  * eps=1e-8 is a fp32 no-op since v_cum >= 1e6 (ulp >= 1/16).

Per-core dataflow, per [128, 4096] tile (16 tiles per core):
  DMA  : load stock tile, load volume tile          (2 MiB each, contiguous,
                                                     issued from nc.sync)
  ACT  : save col 0 of stock (the t==0 fix is an exact copy of s0)
  DVE  : pv_cum = custom scan(ADD, s*v)             (~4.3 us, in place)
  DVE  : v_cum  = custom scan(ADD, v)               (~4.2 us, in place)
  ACT  : r = exp(-ln(v_cum))                        (dedicated ln table set:
                                                     reloads cost ~2.7us/tile
                                                     but its spline is ~10x
                                                     more accurate than the
                                                     combined ln+exp set)
  DVE  : vwap = pv_cum * r, in two halves           (tensor_tensor, ~2.2 us
         each; the first half's store DMA issues mid-tile, keeping the
         store queue busier — measured ~29us/core faster than one store)
  ACT  : restore col 0
  DMA  : store vwap tile halves (issued from nc.scalar — a second DGE
         queue; ~25us/core faster than all-DMAs-on-sync)
Engine busy per core: DVE ~210us, ACT ~220us, DMA ~290us -> DMA-bound.
(n_blk>1 splits each scan into blocks with carries fused into ACT bias /
an (Src0+C0)*Src1 custom op — kept for accuracy headroom but off by
default: the error is ln-spline-dominated either way, and n_blk=1 is
~20us/core faster from the lower instruction count.)
"""

import numpy as np

NUM_PATHS = 16384
TIME = 4096
N_CORES = 8
ROWS = NUM_PATHS // N_CORES  # rows per core
P = 128  # SBUF partitions

_CACHE = {}

_COMBINED_SET = "natural_log_exp_and_others"


def _single_act_set_bacc():
    import concourse.bacc as bacc

    class SingleActSetBacc(bacc.Bacc):
        """Restrict the activation-table-load pass to one set holding
        Ln+Exp+Copy so alternating Ln/Exp doesn't reload tables every tile."""

        def insert_act_table_loads(self):
            import bass_rust
            import concourse.mybir as mybir
            from concourse.hw_specs import get_activation_tables

            has_activation = any(
                isinstance(i, mybir.InstActivation)
                for b in self.main_func.blocks
                for i in b.instructions
            )
            if not has_activation:
                return
            tables = [
                (name, fns if name == _COMBINED_SET else set())
                for name, fns in get_activation_tables(self.m.arch).items()
            ]
            bass_rust.insert_act_table_loads(self, tables)

    return SingleActSetBacc


def _register_custom_ops():
    """Register the two custom DVE cumsum ops (idempotent)."""
    import concourse.dve_ops as dve_ops
    from concourse.dve_ops import DveOp
    from concourse.dve_spec import (
        AluOp, C0, Spec, Src0, Src1, lower, scan, spec_leaves,
    )
    from concourse.dve_uop import DveOpSpec

    def register(name, spec):
        for o in dve_ops.OPS:
            if o.name == name:
                return o
        op = DveOp(name, spec, subdim=False, uops_sha={})
        dve_ops.OPS.append(op)
        dve_ops.CUSTOM_DVE_SPECS[name] = spec
        dve_ops._SUB_OPCODE_FOR_NAME[name] = (
            dve_ops._CUSTOM_DVE_ROW_BASE + len(dve_ops.OPS) - 1
        )
        assert dve_ops._SUB_OPCODE_FOR_NAME[name] < 0x20
        # self-pin the uop hashes (same computation DveOp.compile checks)
        for ver in ("v3", "v4"):
            s = DveOpSpec(
                name=name,
                opcode=dve_ops.get_dve_sub_opcode(name),
                uops=lower(spec, ver=ver),
                rd1_en=Src1 in spec_leaves(spec),
            )
            op.uops_sha[ver] = s.sha(ver)
        return op

    pv = register(
        "PV_CUMSUM_ANT",
        Spec(
            body=scan(AluOp.ADD, Src0 * Src1),
            reference=lambda in0, in1, s0, s1, imm2: np.cumsum(
                in0.astype(np.float32) * in1.astype(np.float32),
                axis=-1, dtype=np.float32,
            ),
        ),
    )
    v = register(
        "V_CUMSUM_ANT",
        Spec(
            body=scan(AluOp.ADD, Src0),
            reference=lambda in0, in1, s0, s1, imm2: np.cumsum(
                in0, axis=-1, dtype=np.float32
            ),
        ),
    )
    addmul = register(
        "ADD_MUL_ANT",  # out = (in0 + s0) * in1; s0 is a [P,1] per-partition AP
        Spec(
            body=(Src0 + C0) * Src1,
            reference=lambda in0, in1, s0, s1, imm2: (
                (in0 + s0) * in1
            ).astype(np.float32),
        ),
    )
    return pv, v, addmul


def _build(rows=ROWS, time=TIME, bufs=4, reps=1, n_blk=1, store_eng="scalar",
           load_split=False, half_store=True, store_splits=2, split2=False,
           recip_mode="act"):
    """recip_mode: 'act' = 1/v_cum via exp(-ln(x)) on ACT (ln table ~5e-5);
    'dve' = reciprocal_approx_fast custom op on DVE (~3e-6, frees ACT);
    'mix' = alternate per tile (balances DVE/ACT engine busy)."""
    """n_blk: scans run per block of time/n_blk columns, with the running
    carry applied via ACT bias (v) / the ADD_MUL custom op (pv).  Shorter
    scan segments cut the fp32 association error of the running sums
    (~proportional to segment length) at no DVE-time cost."""
    import concourse.tile as tile
    import concourse.mybir as mybir

    import concourse.bacc as bacc

    pv_op, v_op, addmul_op = _register_custom_ops()
    # Plain Bacc: the ATL pass alternates the natural_log / exp_and_others
    # table sets (~2.7us per reload), but ACT has headroom under the DMA
    # floor, and the DEDICATED natural_log set's ln spline is ~10x more
    # accurate than the combined natural_log_exp_and_others set's (4.9e-4 vs
    # 5e-5 end-to-end, HW-measured) — accuracy wins here.
    nc = bacc.Bacc("TRN2", target_bir_lowering=False, debug=False)
    f32 = mybir.dt.float32
    stock = nc.dram_tensor("stock_paths", [rows, time], f32, kind="ExternalInput").ap()
    vol = nc.dram_tensor("volume_paths", [rows, time], f32, kind="ExternalInput").ap()
    out = nc.dram_tensor("vwap_out", [rows, time], f32, kind="ExternalOutput").ap()

    Ln = mybir.ActivationFunctionType.Ln
    Exp = mybir.ActivationFunctionType.Exp

    seg = time // n_blk
    n_tiles = rows // P
    with tile.TileContext(nc) as tc:
        with (
            tc.tile_pool(name="big", bufs=bufs) as big,
            tc.tile_pool(name="small", bufs=bufs) as small,
        ):
            for i in range(n_tiles * reps):
                r0 = (i % n_tiles) * P
                ts = big.tile([P, time], f32, tag="ts")
                tv = big.tile([P, time], f32, tag="tv")
                if split2:
                    # two independent half-pipelines per tile: load half,
                    # scan half, ln/exp half, mul half, store half.  Half 1's
                    # running-sum carries are just half 0's end values.
                    h = time // 2
                    st = getattr(nc, store_eng)
                    nc.sync.dma_start(ts[:, :h], stock[r0 : r0 + P, :h])
                    nc.sync.dma_start(tv[:, :h], vol[r0 : r0 + P, :h])
                    nc.sync.dma_start(ts[:, h:], stock[r0 : r0 + P, h:])
                    nc.sync.dma_start(tv[:, h:], vol[r0 : r0 + P, h:])
                    t0 = small.tile([P, 1], f32, tag="t0")
                    nc.scalar.copy(t0[:], ts[:, 0:1])
                    # half 0
                    nc.vector._custom_dve(pv_op, out=ts[:, :h], in0=ts[:, :h],
                                          in1=tv[:, :h])
                    nc.vector._custom_dve(v_op, out=tv[:, :h], in0=tv[:, :h])
                    cp1 = small.tile([P, 1], f32, tag="cp1")
                    cv1 = small.tile([P, 1], f32, tag="cv1")
                    nc.vector.tensor_copy(cp1[:], ts[:, h - 1 : h])
                    nc.vector.tensor_copy(cv1[:], tv[:, h - 1 : h])
                    nc.scalar.activation(tv[:, :h], tv[:, :h], Ln)
                    nc.scalar.activation(tv[:, :h], tv[:, :h], Exp, scale=-1.0)
                    nc.vector.tensor_mul(ts[:, :h], ts[:, :h], tv[:, :h])
                    nc.scalar.copy(ts[:, 0:1], t0[:])
                    st.dma_start(out[r0 : r0 + P, :h], ts[:, :h])
                    # half 1 (carries via ln bias / addmul s0)
                    nc.vector._custom_dve(pv_op, out=ts[:, h:], in0=ts[:, h:],
                                          in1=tv[:, h:])
                    nc.vector._custom_dve(v_op, out=tv[:, h:], in0=tv[:, h:])
                    nc.scalar.activation(tv[:, h:], tv[:, h:], Ln, bias=cv1[:])
                    nc.scalar.activation(tv[:, h:], tv[:, h:], Exp, scale=-1.0)
                    nc.vector._custom_dve(addmul_op, out=ts[:, h:],
                                          in0=ts[:, h:], in1=tv[:, h:],
                                          s0=cp1[:])
                    st.dma_start(out[r0 : r0 + P, h:], ts[:, h:])
                    continue
                nc.sync.dma_start(ts[:], stock[r0 : r0 + P, :])
                vol_eng = nc.scalar if load_split else nc.sync
                vol_eng.dma_start(tv[:], vol[r0 : r0 + P, :])
                t0 = small.tile([P, 1], f32, tag="t0")
                nc.scalar.copy(t0[:], ts[:, 0:1])
                # block-local scans (pv before v per block: pv reads raw v)
                for b in range(n_blk):
                    sl = slice(b * seg, (b + 1) * seg)
                    nc.vector._custom_dve(pv_op, out=ts[:, sl], in0=ts[:, sl],
                                          in1=tv[:, sl])
                    nc.vector._custom_dve(v_op, out=tv[:, sl], in0=tv[:, sl])
                if n_blk > 1:
                    # carries: inclusive cumsum of the block-end values
                    cp = small.tile([P, n_blk], f32, tag="cp")
                    cv = small.tile([P, n_blk], f32, tag="cv")
                    ends = slice(seg - 1, time, seg)
                    nc.vector.tensor_copy(cp[:], ts[:, ends])
                    nc.vector.tensor_copy(cv[:], tv[:, ends])
                    nc.vector._custom_dve(v_op, out=cp[:], in0=cp[:])
                    nc.vector._custom_dve(v_op, out=cv[:], in0=cv[:])
                # v_cum -> 1/v_cum
                use_dve_recip = recip_mode == "dve" or (
                    recip_mode == "mix" and i % 2 == 0)
                if use_dve_recip and n_blk == 1:
                    nc.vector.reciprocal_approx_fast(out=tv[:], in_=tv[:])
                else:
                    for b in range(n_blk):
                        sl = slice(b * seg, (b + 1) * seg)
                        bias = 0.0 if b == 0 else cv[:, b - 1 : b]
                        nc.scalar.activation(tv[:, sl], tv[:, sl], Ln, bias=bias)
                    nc.scalar.activation(tv[:], tv[:], Exp, scale=-1.0)
                # vwap = (pv_cum_block + pv_carry) * r
                if load_split:
                    # balance the two HWDGE queues: 3 MiB/tile each
                    st = nc.sync if i % 2 == 0 else nc.scalar
                else:
                    st = getattr(nc, store_eng)
                if half_store and n_blk == 1:
                    # split mul+store so the store queue starts mid-tile
                    w = time // store_splits
                    for k in range(store_splits):
                        sl = slice(k * w, (k + 1) * w)
                        nc.vector.tensor_mul(ts[:, sl], ts[:, sl], tv[:, sl])
                        if k == 0:
                            nc.scalar.copy(ts[:, 0:1], t0[:])
                        st.dma_start(out[r0 : r0 + P, sl], ts[:, sl])
                else:
                    for b in range(n_blk):
                        sl = slice(b * seg, (b + 1) * seg)
                        if b == 0:
                            nc.vector.tensor_mul(ts[:, sl], ts[:, sl], tv[:, sl])
                        else:
                            nc.vector._custom_dve(
                                addmul_op, out=ts[:, sl], in0=ts[:, sl],
                                in1=tv[:, sl], s0=cp[:, b - 1 : b],
                            )
                    nc.scalar.copy(ts[:, 0:1], t0[:])
                    st.dma_start(out[r0 : r0 + P, :], ts[:])
    nc.compile()
    return nc


def _get_nc():
    if "nc" not in _CACHE:
        _CACHE["nc"] = _build()
    return _CACHE["nc"]


def kernel(stock_paths: np.ndarray, volume_paths: np.ndarray) -> np.ndarray:
    from concourse.bass_utils import run_bass_kernel_spmd

    stock_paths = np.ascontiguousarray(stock_paths, dtype=np.float32)
    volume_paths = np.ascontiguousarray(volume_paths, dtype=np.float32)
    assert stock_paths.shape == (NUM_PATHS, TIME)

    nc = _get_nc()
    in_maps = [
        {
            "stock_paths": stock_paths[i * ROWS : (i + 1) * ROWS],
            "volume_paths": volume_paths[i * ROWS : (i + 1) * ROWS],
        }
        for i in range(N_CORES)
    ]
    res = run_bass_kernel_spmd(nc, in_maps, core_ids=list(range(N_CORES)))
    return np.concatenate([r["vwap_out"] for r in res.results], axis=0)
